# revision 26
# baseline (speedup 1.0000x reference)
"""Complex-magnitude MaxPool2d (k=2, s=2) Trainium2 Bass kernel.

Input  x:  [16, 2, 64, 224, 224] f32  (plane 0 = real, plane 1 = imag)
Output:    [16, 2, 64, 112, 112] f32  (value of the window element with the
                                       largest |z|^2 = re^2 + im^2)

Sharding: pure data parallel over batch: 16 / 8 cores = 2 examples per core.
Per core the 2(batch) x 64(channel) = 128 image planes map 1:1 onto the 128
SBUF partitions.

Layout: the host de-interleaves each 224x224 plane into its four 2x2-window
quadrants and interleaves (re,im) per pixel: per partition [q, ho, wo, ri]
(q=0..3 is the window position in argmax order TL,TR,BL,BR).  Every chunk is
4 contiguous 7 KiB runs per partition and every engine op is a dense AP.

Selection reproduces jnp.argmax's first-index tie-break exactly via a
tournament with >= at each stage (TL vs TR, BL vs BR, then top vs bottom)
on f32-exact norms.  Winners are written in place into the loser quadrant's
plane, so selects need no pre-fill copies.

Engine split (GPSIMD stays idle: any Pool op mutually blocks DVE 2-stream
ops on the shared SBUF port — HW-measured):
  VectorE : three single-uop custom ops (norm2 = re^2+im^2 bit-exact;
            signed-max packing the h-winner side into the sign bit; the
            v-mask via order-exact squared compare), a 2x-eligible
            tensor_scalar sign->mask shift, and two predicated selects on
            bf16 (re,im) pairs packed as int32 elements.  Selection
            DECISIONS stay f32-exact; only output values round to bf16
            (~1.7e-3 rel err, gate is 2e-2).  11 DVE elems per output
            pixel vs the 12-elem plain tournament.
  ScalarE : one contiguous f32 -> bf16 cast per chunk + the store DMA ring
  Sync    : input DMA ring (separate HWDGE ring from stores)
"""

import numpy as np

import concourse.bass as bass
import concourse.mybir as mybir
from concourse import bacc, bass_utils, tile

# Per-core shard geometry (hardcoded; kernel.py must be self-contained).
NCORES = 8
B = 2             # batch per core
RI = 2            # real/imag planes
C = 64            # channels
H = W = 224
HO, WO = H // 2, W // 2
Q = 4             # window quadrants (TL, TR, BL, BR)
P = 128           # SBUF partitions = B * C
CH = 8            # output rows per steady-state chunk
# small warmup chunks let compute start ~8us earlier (first DMA is small)
CHUNKS = [4, 4] + [CH] * ((HO - 8) // CH)

F32 = mybir.dt.float32
BF16 = mybir.dt.bfloat16
U32 = mybir.dt.uint32
I32 = mybir.dt.int32
OP = mybir.AluOpType

_NC_CACHE = []


def _register(name, spec):
    """Register (once) a custom DVE op; sha computed locally (lower() is
    deterministic), row appended to the production table."""
    import concourse.dve_ops as dops
    from concourse.dve_spec import lower, _has_src1
    from concourse.dve_uop import DveOpSpec

    for o in dops.OPS:
        if o.name == name:
            return o
    row = dops._CUSTOM_DVE_ROW_BASE + len(dops.OPS)
    shas = {}
    for ver in ("v3", "v4"):
        u = lower(spec, ver=ver)
        shas[ver] = DveOpSpec(
            name=name, opcode=row, uops=u, rd1_en=_has_src1(spec)
        ).sha(ver)
    op = dops.DveOp(name, spec, subdim=False, uops_sha=shas)
    dops.OPS.append(op)
    dops.CUSTOM_DVE_SPECS[name] = spec
    dops._SUB_OPCODE_FOR_NAME[name] = row
    return op


def _custom_ops():
    """Three single-uop custom DVE ops:
    - NORM2: re^2 + im^2, IEEE f32 mul/mul/add — bit-exact vs the reference.
    - EMAX: select(n0>=n1, -n0, n1) — winner norm with the winner SIDE packed
      into the sign bit (norms are >=0, the sign is free).  Left/tie gives
      -n0 (sign set, including -0.0 on 0-0 ties); right-strict gives +n1 > 0.
      One op replaces the is_ge + max pair.
    - VMASK: select(sq(a)>=sq(b), s0, 0) — squaring strips the sign-bit flag,
      and fl(x^2) is strictly monotone on normal f32 (b>=a+ulp =>
      b^2-a^2 >= 2ulp(a^2)), so the compare is order-exact.
    """
    from concourse.dve_spec import Spec, Src0, Src1, Zero, C0, sq, select

    norm2 = _register(
        "COMPLEX_NORM2_ANT",
        Spec(
            body=sq(Src0) + sq(Src1),
            reference=lambda in0, in1, s0, s1, imm2: (
                in0.astype(np.float32) * in0 + in1.astype(np.float32) * in1
            ),
        ),
    )
    emax = _register(
        "SIGNED_MAX_ANT",
        Spec(
            body=select(Src0 >= Src1, Zero - Src0, Src1),
            reference=lambda in0, in1, s0, s1, imm2: np.where(
                in0 >= in1, -in0.astype(np.float32), in1
            ).astype(np.float32),
        ),
    )
    vmask = _register(
        "SQ_GE_MASK_ANT",
        Spec(
            body=select(sq(Src0) >= sq(Src1), C0, Zero),
            reference=lambda in0, in1, s0, s1, imm2: np.where(
                in0.astype(np.float32) * in0 >= in1.astype(np.float32) * in1,
                np.float32(s0),
                np.float32(0),
            ).astype(np.float32),
        ),
    )
    return norm2, emax, vmask


def _build_nc() -> bass.Bass:
    norm2, emax, vmask = _custom_ops()
    nc = bacc.Bacc("TRN2", target_bir_lowering=False, debug=False)
    # host pre-quadrantized, (re,im)-interleaved: [b*c, q, ho, wo, ri]
    x = nc.dram_tensor("x", [P, Q, HO, WO, RI], F32, kind="ExternalInput").ap()
    # interleaved (re,im) bf16 output; host de-interleaves + upcasts
    out = nc.dram_tensor("out", [P, HO, WO, RI], BF16, kind="ExternalOutput").ap()

    with tile.TileContext(nc) as tc:
        with tc.tile_pool(name="pool", bufs=2) as pool:
            r0 = 0
            for ch in CHUNKS:
                npix = ch * WO
                xin = pool.tile([P, Q * npix * RI], F32, tag="xin", bufs=5)
                nc.sync.dma_start(
                    out=xin.rearrange(
                        "p (q r w ri) -> p q r w ri", q=Q, r=ch, w=WO, ri=RI
                    ),
                    in_=x[:, :, r0 : r0 + ch],
                )

                # bf16 value planes, same pair-interleaved layout (contiguous
                # cast on ScalarE, independent of the norm pass)
                xb = pool.tile([P, Q * npix * RI], BF16, tag="xb")
                nc.scalar.copy(out=xb, in_=xin)

                # norm2 in one fused DVE pass; strided (re,im) pair reads
                nrm = pool.tile([P, Q * npix], F32, tag="nrm", bufs=2)
                xpair = xin.rearrange("p (n ri) -> p n ri", ri=RI)
                nc.vector._custom_dve(
                    norm2, out=nrm, in0=xpair[:, :, 0], in1=xpair[:, :, 1]
                )
                nrm3 = nrm.rearrange("p (q m) -> p q m", q=Q)
                nrm_i = nrm.bitcast(I32).rearrange("p (q m) -> p q m", q=Q)

                # signed-max: winner norm with the left-wins flag in the sign
                # bit, in place over the odd-quadrant norm slots
                nc.vector._custom_dve(
                    emax,
                    out=nrm3[:, 1::2],
                    in0=nrm3[:, 0::2],
                    in1=nrm3[:, 1::2],
                )
                # vertical mask (1.0/0.0) from the squared signed-maxes, into
                # the dead even-norm slot; top wins ties
                nc.vector._custom_dve(
                    vmask, out=nrm3[:, 0], in0=nrm3[:, 1], in1=nrm3[:, 3], s0=1.0
                )
                # horizontal masks: sign -> all-ones (left wins, incl -0.0
                # ties), in place; single-src tensor_scalar can run 2x
                nc.vector.tensor_scalar(
                    out=nrm_i[:, 1::2],
                    in0=nrm_i[:, 1::2],
                    scalar1=31,
                    scalar2=None,
                    op0=OP.arith_shift_right,
                )

                # selects of the packed (re,im) bf16 pairs, in place
                xb3 = xb.bitcast(U32).rearrange("p (q m) -> p q m", q=Q)
                nc.vector.copy_predicated(
                    out=xb3[:, 1::2], mask=nrm_i[:, 1::2], data=xb3[:, 0::2]
                )
                nc.vector.copy_predicated(
                    out=xb3[:, 3], mask=nrm_i[:, 0], data=xb3[:, 1]
                )

                # winner plane q=3 is the contiguous bf16 tail -> store on the
                # Scalar HWDGE ring (separate from the input ring)
                nc.scalar.dma_start(
                    out=out[:, r0 : r0 + ch].rearrange("p r w ri -> p (r w ri)"),
                    in_=xb[:, 3 * npix * RI :],
                )
                r0 += ch
    nc.compile()
    return nc


def get_nc() -> bass.Bass:
    if not _NC_CACHE:
        _NC_CACHE.append(_build_nc())
    return _NC_CACHE[0]


def kernel(x: np.ndarray, **run_kwargs) -> np.ndarray:
    nc = get_nc()
    xs = np.asarray(x, dtype=np.float32)
    assert xs.shape == (NCORES * B, RI, C, H, W), xs.shape
    # [b, ri, c, 2ho+dy, 2wo+dx] -> [b, c, dy, dx, ho, wo, ri]
    xr = xs.reshape(NCORES * B, RI, C, HO, 2, WO, 2)
    xt = np.ascontiguousarray(xr.transpose(0, 2, 4, 6, 3, 5, 1)).reshape(
        NCORES * B, C, Q, HO, WO, RI
    )
    in_maps = [
        {"x": xt[B * i : B * (i + 1)].reshape(P, Q, HO, WO, RI)}
        for i in range(NCORES)
    ]
    res = bass_utils.run_bass_kernel_spmd(
        nc, in_maps, core_ids=list(range(NCORES)), **run_kwargs
    )
    # per-core [128, ho, wo, ri] bf16 -> [b, c, ho, wo, ri] -> [b, ri, c, ho, wo]
    out = np.concatenate(
        [
            np.asarray(res.results[i]["out"])
            .astype(np.float32)
            .reshape(B, C, HO, WO, RI)
            .transpose(0, 4, 1, 2, 3)
            for i in range(NCORES)
        ],
        axis=0,
    )
    if run_kwargs:
        kernel.last_results = res
    return np.ascontiguousarray(out)
